# revision 3
# baseline (speedup 1.0000x reference)
"""Trainium2 Bass kernel for CTC loss (nn_CTCLayer).

Inputs (full, unsharded):
  y_true       [64, 48]  int32  labels (blank excluded)
  y_pred       [64, 128, 4000] float32 probabilities
  label_length [64, 1]  int32
Output: loss [64, 1] float32  (= tf.keras ctc_batch_cost, input_length == T)

Strategy (pure data parallelism, 8 examples per core on 8 cores):

The CTC forward DP over S = 2L+1 = 97 extended states only ever reads
y_pred at the (<= L+1) classes present in each example's extended label
sequence. So instead of streaming all of y_pred, each core gathers the
needed columns with one indirect DMA per example (offset tables computed
on the host from y_true during sharding) into a [128, T, 8] SBUF tile
(states on partitions, time and example on the free axis).

The DP itself runs in the scaled probability domain:
    U_t = (kappa * Aaug^T @ U_{t-1}) * paug[:, t, :]
where Aaug is a STATIC [128,128] transition matrix shared by all 8
examples of a core (one 128x128xN=8 matmul + one [128,8] vector multiply
per timestep). Rows 97..127 of U are auxiliary "W" rows that implement
the forbidden-skip correction for adjacent repeated labels exactly
(bit-exact cancellation; see below). Dynamic range is handled by the
constant per-step scale kappa plus 3 sum-renormalizations whose factors
are re-applied in log space at the end.

Repeated-label correction: CTC forbids the skip s-2 -> s when
ext[s] == ext[s-2]. The shared transition matrix allows all odd skips;
for each actual repeat (example b_i, state s_i) an aux row 97+i tracks
W_i = U[s_i - 2] for example b_i only (its gathered probability row is a
copy of row s_i-2's, and other examples' columns are OOB-skipped in the
gather so they stay ~0), and column s_i carries -kappa at row 97+i,
cancelling the forbidden contribution bit-exactly (identical matmul
summation order and identical multiplier bits).

Padding states s > 2*label_length never influence the read-out states
(transitions are monotone in s), and their gather rows are OOB-skipped
so they hold ~EPS and cannot pollute the dynamic range.
"""

import os
import sys
import math

import numpy as np

if "/opt/trn_rl_repo" not in sys.path:
    sys.path.insert(0, "/opt/trn_rl_repo")

# ---------------------------------------------------------------- constants
B, T, C, L = 64, 128, 4000, 48
S = 2 * L + 1            # 97 extended states
P = 128                  # partitions / augmented state count
R = P - S                # 31 aux correction rows
NCORES = 8
BSH = B // NCORES        # 8 examples per core
BLANK = C - 1
EPS = 1e-7               # keras backend epsilon (reference adds before log)
KAPPA = 2048.0           # per-step scale folded into the transition matrix
RENORM_TS = (32, 64, 96)
NRE = len(RENORM_TS)
SENTINEL = 1_000_000_000  # OOB row index -> indirect DMA skips the row
NROWS = BSH * C          # rows of the per-core transposed y_pred table

# consts tensor column layout (f32 [128, CW])
COL_A = 0                # [0:128]    Aaug (lhsT: [k, m] = kappa * trans k->m)
COL_IM = 128             # [128:136]  init mask [128, 8]
COL_FS = 136             # [136:144]  final selector [128, 8]
COL_ONE = 144            # [144:145]  ones column (renorm sums)
COL_I8 = 145             # [145:153]  8x8 identity (rows 0..7)
COL_BR = 153             # [153:281]  row 0 = ones row (renorm broadcast)
CW = 281

_CACHE = {}


# ---------------------------------------------------------------- host tables
def _build_core_tables(y_true, label_length):
    """Static per-core data. Returns (offs [128, 8] int32, consts [128, CW] f32,
    n_dropped_corrections)."""
    n = y_true.shape[0]
    ll = label_length.reshape(-1).astype(np.int64)
    lab = np.where(np.arange(L)[None, :] < ll[:, None], y_true.astype(np.int64), BLANK)
    ext = np.full((n, S), BLANK, dtype=np.int64)
    ext[:, 1::2] = lab

    aug = []  # (i, b, s_i)
    for b in range(n):
        for s_i in range(3, int(min(2 * ll[b] - 1, S - 1)) + 1, 2):
            j = (s_i - 1) // 2
            if lab[b, j] == lab[b, j - 1]:
                aug.append((len(aug), b, s_i))
    dropped = max(0, len(aug) - R)
    aug = aug[:R]

    offs = np.full((P, BSH), SENTINEL, dtype=np.int32)
    for b in range(n):
        live = 2 * ll[b]  # states 0..2*ll inclusive are live
        for s in range(min(live, S - 1) + 1):
            offs[s, b] = b * C + ext[b, s]
    for (i, b, s_i) in aug:
        offs[S + i, b] = b * C + ext[b, s_i - 2]

    A = np.zeros((P, P), dtype=np.float64)
    for m in range(S):
        A[m, m] = 1.0
        if m >= 1:
            A[m - 1, m] = 1.0
        if m >= 2 and (m % 2 == 1):
            A[m - 2, m] = 1.0
    for (i, b, s_i) in aug:
        A[S + i, s_i] = -1.0
    for (i, b, s_i) in aug:
        A[:, S + i] = A[:, s_i - 2]

    consts = np.zeros((P, CW), dtype=np.float32)
    consts[:, COL_A:COL_A + P] = (A * KAPPA).astype(np.float32)
    consts[0, COL_IM:COL_IM + BSH] = 1.0
    consts[1, COL_IM:COL_IM + BSH] = 1.0
    for (i, b, s_i) in aug:
        if s_i == 3:
            consts[S + i, COL_IM + b] = 1.0
    for b in range(n):
        consts[2 * ll[b], COL_FS + b] = 1.0
        consts[2 * ll[b] - 1, COL_FS + b] = 1.0
    consts[:, COL_ONE] = 1.0
    for d in range(BSH):
        consts[d, COL_I8 + d] = 1.0
    consts[0, COL_BR:COL_BR + P] = 1.0
    return offs, consts, dropped


# ---------------------------------------------------------------- host fallback
def _host_ctc(y_true_b, y_pred_b, ll_b):
    """Exact log-domain port of the reference for one example (float64)."""
    NEG = -1e30
    ll = int(ll_b)
    lab = np.where(np.arange(L) < ll, y_true_b.astype(np.int64), BLANK)
    ext = np.full((S,), BLANK, dtype=np.int64)
    ext[1::2] = lab
    logp = np.log(y_pred_b.astype(np.float64) + EPS)          # [T, C]
    lp = logp[:, ext]                                         # [T, S]
    ext_m2 = np.concatenate([[BLANK, BLANK], ext[:-2]])
    allow = (ext != BLANK) & (ext != ext_m2)
    alpha = np.where(np.arange(S) < 2, lp[0], NEG)
    for t in range(1, T):
        a0 = alpha
        a1 = np.concatenate([[NEG], alpha[:-1]])
        a2 = np.where(allow, np.concatenate([[NEG, NEG], alpha[:-2]]), NEG)
        m = np.maximum(np.maximum(a0, a1), a2)
        alpha = m + np.log(np.exp(a0 - m) + np.exp(a1 - m) + np.exp(a2 - m)) + lp[t]
    ab, al = alpha[2 * ll], alpha[2 * ll - 1]
    m = max(ab, al)
    return -(m + math.log(math.exp(ab - m) + math.exp(al - m)))


# ---------------------------------------------------------------- bass program
def _build_program():
    import concourse.bacc as bacc
    import concourse.bass as bass
    import concourse.tile as tile
    import concourse.mybir as mybir

    nc = bacc.Bacc("TRN2", target_bir_lowering=False, debug=False,
                   enable_asserts=False, num_devices=NCORES)
    ypt_d = nc.dram_tensor("ypt", [NROWS, T], mybir.dt.float32, kind="ExternalInput")
    offs_d = nc.dram_tensor("offs", [P, BSH], mybir.dt.int32, kind="ExternalInput")
    consts_d = nc.dram_tensor("consts", [P, CW], mybir.dt.float32, kind="ExternalInput")
    loss_d = nc.dram_tensor("loss", [1, BSH], mybir.dt.float32, kind="ExternalOutput")

    fp32 = mybir.dt.float32
    mult = mybir.AluOpType.mult

    with tile.TileContext(nc) as tc:
        with (
            tc.tile_pool(name="cpool", bufs=1) as cpool,
            tc.tile_pool(name="upool", bufs=2) as upool,
            tc.tile_pool(name="spool", bufs=1) as spool,
            tc.tile_pool(name="ps", bufs=2, space="PSUM") as ps,
            tc.tile_pool(name="ps_small", bufs=2, space="PSUM") as ps_small,
        ):
            consts = cpool.tile([P, CW], fp32, tag="consts")
            nc.sync.dma_start(consts[:], consts_d[:])
            offs = cpool.tile([P, BSH], mybir.dt.int32, tag="offs")
            nc.sync.dma_start(offs[:], offs_d[:])
            paug = cpool.tile([P, BSH, T], fp32, tag="paug")
            nc.gpsimd.memset(paug[:], 0.0)
            for b in range(BSH):
                nc.gpsimd.indirect_dma_start(
                    out=paug[:, b, :],
                    out_offset=None,
                    in_=ypt_d[:],
                    in_offset=bass.IndirectOffsetOnAxis(ap=offs[:, b:b + 1], axis=0),
                    bounds_check=NROWS - 1,
                    oob_is_err=False,
                )
            # reference takes log(y_pred + EPS); prob domain -> add EPS once
            nc.vector.tensor_scalar_add(paug[:], paug[:], EPS)

            A_ap = consts[:, COL_A:COL_A + P]
            norms = spool.tile([1, NRE * BSH], fp32, tag="norms")

            U = upool.tile([P, BSH], fp32, tag="U")
            nc.vector.tensor_tensor(
                out=U[:], in0=paug[:, :, 0], in1=consts[:, COL_IM:COL_IM + BSH],
                op=mult)

            ri = 0
            for t in range(1, T):
                stp = ps.tile([P, BSH], fp32, tag="step")
                nc.tensor.matmul(stp[:], A_ap, U[:], start=True, stop=True)
                U = upool.tile([P, BSH], fp32, tag="U")
                nc.vector.tensor_tensor(
                    out=U[:], in0=stp[:], in1=paug[:, :, t], op=mult)
                if t in RENORM_TS:
                    nm = ps_small.tile([1, BSH], fp32, tag="sm")
                    nc.tensor.matmul(nm[:], consts[:, COL_ONE:COL_ONE + 1], U[:],
                                     start=True, stop=True)
                    rrow = norms[0:1, ri * BSH:(ri + 1) * BSH]
                    nc.vector.reciprocal(rrow, nm[:])
                    bc = ps.tile([P, BSH], fp32, tag="bc")
                    nc.tensor.matmul(bc[:], consts[0:1, COL_BR:COL_BR + P], rrow,
                                     start=True, stop=True)
                    U2 = upool.tile([P, BSH], fp32, tag="U")
                    nc.vector.tensor_tensor(out=U2[:], in0=U[:], in1=bc[:], op=mult)
                    U = U2
                    ri += 1

            # tail: fin[b] = U[2ll, b] + U[2ll-1, b] via selector matmul + diag
            fin_ps = ps_small.tile([BSH, BSH], fp32, tag="sm")
            nc.tensor.matmul(fin_ps[:], U[:], consts[:, COL_FS:COL_FS + BSH],
                             start=True, stop=True)
            masked = spool.tile([BSH, BSH], fp32, tag="masked")
            nc.vector.tensor_tensor(
                out=masked[:], in0=fin_ps[:],
                in1=consts[0:BSH, COL_I8:COL_I8 + BSH], op=mult)
            fin = spool.tile([BSH, 1], fp32, tag="fin_sb")
            nc.vector.reduce_sum(fin[:], masked[:], axis=mybir.AxisListType.X)
            lnfin = spool.tile([BSH, 1], fp32, tag="lnfin")
            nc.scalar.activation(lnfin[:], fin[:], mybir.ActivationFunctionType.Ln)
            lnT = ps_small.tile([1, BSH], fp32, tag="sm")
            nc.tensor.matmul(lnT[:], lnfin[:], consts[0:BSH, COL_I8:COL_I8 + BSH],
                             start=True, stop=True)
            lnrec = spool.tile([1, NRE * BSH], fp32, tag="lnrec")
            nc.scalar.activation(lnrec[:], norms[:], mybir.ActivationFunctionType.Ln)
            lnrsum = spool.tile([1, BSH], fp32, tag="lnrsum")
            nc.vector.reduce_sum(
                lnrsum[:],
                lnrec[0:1, :].rearrange("p (j b) -> p b j", j=NRE),
                axis=mybir.AxisListType.X)
            total = spool.tile([1, BSH], fp32, tag="total")
            nc.vector.tensor_tensor(out=total[:], in0=lnrsum[:], in1=lnT[:],
                                    op=mybir.AluOpType.subtract)
            loss_row = spool.tile([1, BSH], fp32, tag="loss_row")
            nc.vector.tensor_scalar_add(loss_row[:], total[:],
                                        float((T - 1) * math.log(KAPPA)))
            nc.sync.dma_start(loss_d[:], loss_row[:])

    nc.compile()
    return nc


def _get_program():
    if "nc" not in _CACHE:
        _CACHE["nc"] = _build_program()
    return _CACHE["nc"]


# ---------------------------------------------------------------- entry point
def kernel(y_true: np.ndarray, y_pred: np.ndarray, label_length: np.ndarray) -> np.ndarray:
    from concourse.bass_utils import run_bass_kernel_spmd

    y_true = np.asarray(y_true)
    y_pred = np.asarray(y_pred, dtype=np.float32)
    label_length = np.asarray(label_length)
    assert y_true.shape == (B, L) and y_pred.shape == (B, T, C), (
        f"unexpected shapes {y_true.shape} {y_pred.shape}")

    # host sharding: transpose each example's [T, C] to [C, T] so the device
    # gather reads contiguous per-class rows
    ypt = np.ascontiguousarray(y_pred.transpose(0, 2, 1))  # [B, C, T]

    in_maps = []
    fallback_cores = []
    for core in range(NCORES):
        sl = slice(core * BSH, (core + 1) * BSH)
        offs, consts, dropped = _build_core_tables(y_true[sl], label_length[sl])
        if dropped:
            fallback_cores.append(core)
        in_maps.append({
            "ypt": ypt[sl].reshape(NROWS, T),
            "offs": offs,
            "consts": consts,
        })

    nc = _get_program()
    res = run_bass_kernel_spmd(
        nc, in_maps, core_ids=list(range(NCORES)),
        trace=bool(int(os.environ.get("CTC_TRACE", "0"))),
    )
    _CACHE["last_result"] = res

    loss = np.zeros((B, 1), dtype=np.float32)
    for core in range(NCORES):
        loss[core * BSH:(core + 1) * BSH, 0] = res.results[core]["loss"][0]

    for core in fallback_cores:  # >R adjacent repeats on a core (pathological)
        for b in range(BSH):
            g = core * BSH + b
            loss[g, 0] = _host_ctc(y_true[g], y_pred[g], label_length.reshape(-1)[g])
    return loss


# revision 5
# speedup vs baseline: 2.2720x; 2.2720x over previous
"""Trainium2 Bass kernel for CTC loss (nn_CTCLayer).

Inputs (full, unsharded):
  y_true       [64, 48]  int32  labels (blank excluded)
  y_pred       [64, 128, 4000] float32 probabilities
  label_length [64, 1]  int32
Output: loss [64, 1] float32  (= tf.keras ctc_batch_cost, input_length == T)

Strategy (pure data parallelism, 8 examples per core on 8 cores):

The CTC forward DP over S = 2L+1 = 97 extended states only ever reads
y_pred at the (<= L+1) classes present in each example's extended label
sequence, so each core gathers just those columns with one indirect DMA
per example (row-offset tables computed on the host from y_true during
sharding; y_pred is resharded host-side to [example*class, T] so class
rows are contiguous). The kernel never streams the full y_pred.

The DP runs in the scaled probability domain with states on partitions
and examples on the free axis. To halve the serial depth and keep both
the PE and DVE busy, the forward recursion (t = 0..63) and the backward
recursion (t = 127..64) run as two interleaved chains that meet at
t* = 63, where  P(l|x) = sum_s alpha_t*[s] * beta_t*[s]:

    fwd:  U_t = (F^T @ U_{t-1}) * p[:, :, t]      (matmul -> multiply)
    bwd:  V_t = G_{t} * p[:, :, t];  G_{t-1} = Bw^T @ V_t   (multiply -> matmul)

F and Bw are static per-core [128,128] bf16 matrices with entries
+-kappa (kappa = 2048 keeps products in fp32 range; exactly
representable in bf16). One sum-renormalization per chain (factors
re-applied in log space at the end) bounds the remaining drift.

Rows 97..111 (fwd) and 112..127 (bwd) are auxiliary "W" rows that
correct the forbidden skip transition s-2 -> s when ext[s] == ext[s-2]
(adjacent repeated labels): aux row i tracks the would-be-forbidden
contribution for its example only (its gathered probability row is a
copy of the relevant state's row; other examples' entries are
OOB-skipped in the gather), and the transition matrix subtracts it
where the skip is forbidden. The cancellation is bit-exact because the
aux row's matmul column is a copy of the source state's column and its
multiplier bits are identical. Pathological inputs with more repeats
than aux rows fall back to an exact host computation.

Padding states s > 2*label_length never influence the result states
(transitions are monotone in s) and their gather rows are OOB-skipped.
"""

import os
import sys
import math

import numpy as np

if "/opt/trn_rl_repo" not in sys.path:
    sys.path.insert(0, "/opt/trn_rl_repo")

# ---------------------------------------------------------------- constants
B, T, C, L = 64, 128, 4000, 48
S = 2 * L + 1            # 97 extended states
P = 128                  # partitions
RF = 15                  # fwd aux rows: partitions 97..111
RB = 16                  # bwd aux rows: partitions 112..127
NCORES = 8
BSH = B // NCORES        # 8 examples per core
BLANK = C - 1
EPS = 1e-7               # keras backend epsilon (reference adds before log)
KAPPA = 2048.0
TSTAR = 63               # fwd covers t=0..63, bwd covers t=127..64
RENORM_F = (32, 63)   # 63: normalize U before the meet (product must not underflow)
RENORM_B = (95, 64)   # 64: normalize V in the last bwd round for the same reason
NRE = len(RENORM_F) + len(RENORM_B)
SENTINEL = 1_000_000_000
NROWS = BSH * C

# fp32 consts [128, CW] column layout
COL_IM = 0               # [0:8]     fwd init mask
COL_EM = 8               # [8:16]    bwd init (end-state indicator incl aux copies)
COL_ONE = 16             # [16:17]   fp32 ones column (final sum)
COL_BR = 17              # [17:145]  row 0 = ones row (renorm broadcast)
CW = 145
# bf16 consts [128, 2P+1]: [0:128] F, [128:256] Bw, [256] ones col (renorm sums)
CBW = 2 * P + 1

_CACHE = {}


# ---------------------------------------------------------------- host tables
def _build_core_tables(y_true, label_length):
    """Returns (offs [128,8] i32, constf [128,CW] f32, constb [128,CBW] bf16,
    overflow: bool)."""
    import ml_dtypes
    n = y_true.shape[0]
    ll = label_length.reshape(-1).astype(np.int64)
    lab = np.where(np.arange(L)[None, :] < ll[:, None], y_true.astype(np.int64), BLANK)
    ext = np.full((n, S), BLANK, dtype=np.int64)
    ext[:, 1::2] = lab

    aug = []  # (i, b, s_i): repeat at odd state s_i (skip s_i-2 -> s_i forbidden)
    for b in range(n):
        for s_i in range(3, int(min(2 * ll[b] - 1, S - 1)) + 1, 2):
            j = (s_i - 1) // 2
            if lab[b, j] == lab[b, j - 1]:
                aug.append((len(aug), b, s_i))
    overflow = len(aug) > min(RF, RB)
    aug = aug[:min(RF, RB)]

    offs = np.full((P, BSH), SENTINEL, dtype=np.int32)
    for b in range(n):
        live = int(min(2 * ll[b], S - 1))
        for s in range(live + 1):
            offs[s, b] = b * C + ext[b, s]
    for (i, b, s_i) in aug:
        offs[S + i, b] = b * C + ext[b, s_i - 2]       # fwd aux: p[s_i - 2]
        offs[S + RF + i, b] = b * C + ext[b, s_i]      # bwd aux: p[s_i]

    # forward lhsT: F[k, m] = kappa * allowed(k -> m)
    F = np.zeros((P, P), dtype=np.float64)
    for m in range(S):
        F[m, m] = 1.0
        if m >= 1:
            F[m - 1, m] = 1.0
        if m >= 2 and (m % 2 == 1):
            F[m - 2, m] = 1.0
    for (i, b, s_i) in aug:
        F[S + i, s_i] = -1.0
    for (i, b, s_i) in aug:
        F[:, S + i] = F[:, s_i - 2]

    # backward lhsT: Bw[k, m] = kappa * allowed(m -> k); G_{t-1} = Bw^T @ V_t,
    # V = G * p. Aux row i tracks V[s_i]; subtracted where the skip is forbidden.
    Bw = np.zeros((P, P), dtype=np.float64)
    for k in range(S):
        Bw[k, k] = 1.0
        if k >= 1:
            Bw[k, k - 1] = 1.0
        if k >= 2 and (k % 2 == 1):
            Bw[k, k - 2] = 1.0
    for (i, b, s_i) in aug:
        Bw[S + RF + i, s_i - 2] = -1.0
    for (i, b, s_i) in aug:
        Bw[:, S + RF + i] = Bw[:, s_i]

    constb = np.zeros((P, CBW), dtype=ml_dtypes.bfloat16)
    constb[:, 0:P] = (F * KAPPA).astype(ml_dtypes.bfloat16)
    constb[:, P:2 * P] = (Bw * KAPPA).astype(ml_dtypes.bfloat16)
    constb[:, 2 * P] = ml_dtypes.bfloat16(1.0)

    constf = np.zeros((P, CW), dtype=np.float32)
    constf[0, COL_IM:COL_IM + BSH] = 1.0
    constf[1, COL_IM:COL_IM + BSH] = 1.0
    for (i, b, s_i) in aug:
        if s_i == 3:
            constf[S + i, COL_IM + b] = 1.0
    for b in range(n):
        constf[2 * ll[b], COL_EM + b] = 1.0
        constf[2 * ll[b] - 1, COL_EM + b] = 1.0
    for (i, b, s_i) in aug:
        constf[S + RF + i, COL_EM + b] = constf[s_i, COL_EM + b]
    constf[:, COL_ONE] = 1.0
    constf[0, COL_BR:COL_BR + P] = 1.0
    return offs, constf, constb, overflow


# ---------------------------------------------------------------- host fallback
def _host_ctc(y_true_b, y_pred_b, ll_b):
    """Exact log-domain port of the reference for one example (float64)."""
    NEG = -1e30
    ll = int(ll_b)
    lab = np.where(np.arange(L) < ll, y_true_b.astype(np.int64), BLANK)
    ext = np.full((S,), BLANK, dtype=np.int64)
    ext[1::2] = lab
    lp = np.log(y_pred_b.astype(np.float64) + EPS)[:, ext]    # [T, S]
    ext_m2 = np.concatenate([[BLANK, BLANK], ext[:-2]])
    allow = (ext != BLANK) & (ext != ext_m2)
    alpha = np.where(np.arange(S) < 2, lp[0], NEG)
    for t in range(1, T):
        a0 = alpha
        a1 = np.concatenate([[NEG], alpha[:-1]])
        a2 = np.where(allow, np.concatenate([[NEG, NEG], alpha[:-2]]), NEG)
        m = np.maximum(np.maximum(a0, a1), a2)
        alpha = m + np.log(np.exp(a0 - m) + np.exp(a1 - m) + np.exp(a2 - m)) + lp[t]
    ab, al = alpha[2 * ll], alpha[2 * ll - 1]
    m = max(ab, al)
    return -(m + math.log(math.exp(ab - m) + math.exp(al - m)))


# ---------------------------------------------------------------- bass program
def _build_program():
    import concourse.bacc as bacc
    import concourse.bass as bass
    import concourse.tile as tile
    import concourse.mybir as mybir

    nc = bacc.Bacc("TRN2", target_bir_lowering=False, debug=False,
                   enable_asserts=False, num_devices=NCORES)
    ypt_d = nc.dram_tensor("ypt", [NROWS, T], mybir.dt.float32, kind="ExternalInput")
    offs_d = nc.dram_tensor("offs", [P, BSH], mybir.dt.int32, kind="ExternalInput")
    cf_d = nc.dram_tensor("constf", [P, CW], mybir.dt.float32, kind="ExternalInput")
    cb_d = nc.dram_tensor("constb", [P, CBW], mybir.dt.bfloat16, kind="ExternalInput")
    loss_d = nc.dram_tensor("loss", [1, BSH], mybir.dt.float32, kind="ExternalOutput")

    fp32 = mybir.dt.float32
    bf16 = mybir.dt.bfloat16
    mult = mybir.AluOpType.mult

    with tile.TileContext(nc) as tc:
        with (
            tc.tile_pool(name="cpool", bufs=1) as cpool,
            tc.tile_pool(name="upool", bufs=2) as upool,
            tc.tile_pool(name="spool", bufs=1) as spool,
            tc.tile_pool(name="psf", bufs=2, space="PSUM") as psf,
            tc.tile_pool(name="psb", bufs=2, space="PSUM") as psb,
            tc.tile_pool(name="pss", bufs=1, space="PSUM") as pss,
        ):
            cf = cpool.tile([P, CW], fp32, tag="cf")
            nc.sync.dma_start(cf[:], cf_d[:])
            cb = cpool.tile([P, CBW], bf16, tag="cb")
            nc.sync.dma_start(cb[:], cb_d[:])
            offs = cpool.tile([P, BSH], mybir.dt.int32, tag="offs")
            nc.sync.dma_start(offs[:], offs_d[:])
            paug = cpool.tile([P, BSH, T], fp32, tag="paug")
            nc.gpsimd.memset(paug[:], 0.0)
            for b in range(BSH):
                nc.gpsimd.indirect_dma_start(
                    out=paug[:, b, :],
                    out_offset=None,
                    in_=ypt_d[:],
                    in_offset=bass.IndirectOffsetOnAxis(ap=offs[:, b:b + 1], axis=0),
                    bounds_check=NROWS - 1,
                    oob_is_err=False,
                )
            nc.vector.tensor_scalar_add(paug[:], paug[:], EPS)

            F_ap = cb[:, 0:P]
            Bw_ap = cb[:, P:2 * P]
            onesb = cb[:, 2 * P:2 * P + 1]
            norms = spool.tile([1, NRE * BSH], fp32, tag="norms")
            ri = 0

            def renorm(Z):
                """Divide SBUF bf16 state Z by its per-example column sum."""
                nonlocal ri
                nm = pss.tile([1, BSH], fp32, tag="sm")
                nc.tensor.matmul(nm[:], onesb, Z[:], start=True, stop=True)
                rrow = norms[0:1, ri * BSH:(ri + 1) * BSH]
                nc.vector.reciprocal(rrow, nm[:])
                bc = pss.tile([P, BSH], fp32, tag="bc")
                nc.tensor.matmul(bc[:], cf[0:1, COL_BR:COL_BR + P], rrow,
                                 start=True, stop=True)
                Z2 = upool.tile([P, BSH], bf16, tag="Z2")
                nc.vector.tensor_tensor(out=Z2[:], in0=Z[:], in1=bc[:], op=mult)
                ri += 1
                return Z2

            U = upool.tile([P, BSH], bf16, tag="U")
            nc.vector.tensor_tensor(
                out=U[:], in0=paug[:, :, 0], in1=cf[:, COL_IM:COL_IM + BSH], op=mult)
            gp = None  # bwd chain state (PSUM); first round uses endmask const

            for r in range(1, TSTAR + 2):
                tf_ = r           # fwd timestep this round (valid while <= TSTAR)
                tb = T - r        # bwd multiply timestep this round (127..64)
                # bwd: V = G * p[tb]; G(psum) = Bw^T V
                vin = gp[:] if gp is not None else cf[:, COL_EM:COL_EM + BSH]
                V = upool.tile([P, BSH], bf16, tag="V")
                nc.vector.tensor_tensor(out=V[:], in0=vin, in1=paug[:, :, tb], op=mult)
                if tb in RENORM_B:
                    V = renorm(V)
                gp = psb.tile([P, BSH], fp32, tag="gp")
                nc.tensor.matmul(gp[:], Bw_ap, V[:], start=True, stop=True)
                # fwd: psum = F^T U; U = psum * p[tf]
                if tf_ <= TSTAR:
                    stp = psf.tile([P, BSH], fp32, tag="stp")
                    nc.tensor.matmul(stp[:], F_ap, U[:], start=True, stop=True)
                    U = upool.tile([P, BSH], bf16, tag="U")
                    nc.vector.tensor_tensor(
                        out=U[:], in0=stp[:], in1=paug[:, :, tf_], op=mult)
                    if tf_ in RENORM_F:
                        U = renorm(U)

            # meet: fin[b] = sum_s U_63[s, b] * G_63[s, b] (aux cross-terms vanish:
            # U is zero on bwd-aux rows, G zero on fwd-aux rows)
            prod = spool.tile([P, BSH], fp32, tag="prod")
            nc.vector.tensor_tensor(out=prod[:], in0=U[:], in1=gp[:], op=mult)
            fin = pss.tile([1, BSH], fp32, tag="sm")
            nc.tensor.matmul(fin[:], cf[:, COL_ONE:COL_ONE + 1], prod[:],
                             start=True, stop=True)
            lnfin = spool.tile([1, BSH], fp32, tag="lnfin")
            nc.scalar.activation(lnfin[:], fin[:], mybir.ActivationFunctionType.Ln)
            lnrec = spool.tile([1, NRE * BSH], fp32, tag="lnrec")
            nc.scalar.activation(lnrec[:], norms[:], mybir.ActivationFunctionType.Ln)
            lnrsum = spool.tile([1, BSH], fp32, tag="lnrsum")
            nc.vector.reduce_sum(
                lnrsum[:],
                lnrec[0:1, :].rearrange("p (j b) -> p b j", j=NRE),
                axis=mybir.AxisListType.X)
            total = spool.tile([1, BSH], fp32, tag="total")
            nc.vector.tensor_tensor(out=total[:], in0=lnrsum[:], in1=lnfin[:],
                                    op=mybir.AluOpType.subtract)
            loss_row = spool.tile([1, BSH], fp32, tag="loss_row")
            nc.vector.tensor_scalar_add(loss_row[:], total[:],
                                        float((T - 1) * math.log(KAPPA)))
            nc.sync.dma_start(loss_d[:], loss_row[:])

    nc.compile()
    return nc


def _get_program():
    if "nc" not in _CACHE:
        _CACHE["nc"] = _build_program()
    return _CACHE["nc"]


# ---------------------------------------------------------------- entry point
def kernel(y_true: np.ndarray, y_pred: np.ndarray, label_length: np.ndarray) -> np.ndarray:
    from concourse.bass_utils import run_bass_kernel_spmd

    y_true = np.asarray(y_true)
    y_pred = np.asarray(y_pred, dtype=np.float32)
    label_length = np.asarray(label_length)
    assert y_true.shape == (B, L) and y_pred.shape == (B, T, C), (
        f"unexpected shapes {y_true.shape} {y_pred.shape}")

    # host sharding: transpose each example's [T, C] to [C, T] so the device
    # gather reads contiguous per-class rows
    ypt = np.ascontiguousarray(y_pred.transpose(0, 2, 1))  # [B, C, T]

    in_maps = []
    fallback_cores = []
    for core in range(NCORES):
        sl = slice(core * BSH, (core + 1) * BSH)
        offs, constf, constb, overflow = _build_core_tables(y_true[sl], label_length[sl])
        if overflow:
            fallback_cores.append(core)
        in_maps.append({
            "ypt": ypt[sl].reshape(NROWS, T),
            "offs": offs,
            "constf": constf,
            "constb": constb,
        })

    nc = _get_program()
    res = run_bass_kernel_spmd(
        nc, in_maps, core_ids=list(range(NCORES)),
        trace=bool(int(os.environ.get("CTC_TRACE", "0"))),
    )
    _CACHE["last_result"] = res

    loss = np.zeros((B, 1), dtype=np.float32)
    for core in range(NCORES):
        loss[core * BSH:(core + 1) * BSH, 0] = res.results[core]["loss"][0]

    for core in fallback_cores:  # more repeats than aux rows (pathological)
        for b in range(BSH):
            g = core * BSH + b
            loss[g, 0] = _host_ctc(y_true[g], y_pred[g], label_length.reshape(-1)[g])
    return loss


# revision 10
# speedup vs baseline: 2.3438x; 1.0316x over previous
"""Trainium2 Bass kernel for CTC loss (nn_CTCLayer).

Inputs (full, unsharded):
  y_true       [64, 48]  int32  labels (blank excluded)
  y_pred       [64, 128, 4000] float32 probabilities
  label_length [64, 1]  int32
Output: loss [64, 1] float32  (= tf.keras ctc_batch_cost, input_length == T)

Strategy (pure data parallelism, 8 examples per core on 8 cores):

The CTC forward DP over S = 2L+1 = 97 extended states only ever reads
y_pred at the (<= L+1) classes present in each example's extended label
sequence, so each core gathers just those columns with one indirect DMA
per example (row-offset tables computed on the host from y_true during
sharding; y_pred is resharded host-side to [example*class, T] so class
rows are contiguous). The kernel never streams the full y_pred.

The DP runs in the scaled probability domain with states on partitions
and examples on the free axis. To halve the serial depth and keep both
the PE and DVE busy, the forward recursion (t = 0..63) and the backward
recursion (t = 127..64) run as two interleaved chains that meet at
t* = 63, where  P(l|x) = sum_s alpha_t*[s] * beta_t*[s]:

    fwd:  U_t = (F^T @ U_{t-1}) * p[:, :, t]      (matmul -> multiply)
    bwd:  V_t = G_{t} * p[:, :, t];  G_{t-1} = Bw^T @ V_t   (multiply -> matmul)

F and Bw are static per-core [128,128] bf16 matrices with entries
+-kappa (kappa = 2048 keeps products in fp32 range; exactly
representable in bf16). One sum-renormalization per chain (factors
re-applied in log space at the end) bounds the remaining drift.

Rows 97..111 (fwd) and 112..127 (bwd) are auxiliary "W" rows that
correct the forbidden skip transition s-2 -> s when ext[s] == ext[s-2]
(adjacent repeated labels): aux row i tracks the would-be-forbidden
contribution for its example only (its gathered probability row is a
copy of the relevant state's row; other examples' entries are
OOB-skipped in the gather), and the transition matrix subtracts it
where the skip is forbidden. The cancellation is bit-exact because the
aux row's matmul column is a copy of the source state's column and its
multiplier bits are identical. Pathological inputs with more repeats
than aux rows fall back to an exact host computation.

Padding states s > 2*label_length never influence the result states
(transitions are monotone in s) and their gather rows are OOB-skipped.
"""

import os
import sys
import math

import numpy as np

if "/opt/trn_rl_repo" not in sys.path:
    sys.path.insert(0, "/opt/trn_rl_repo")

# ---------------------------------------------------------------- constants
B, T, C, L = 64, 128, 4000, 48
S = 2 * L + 1            # 97 extended states
P = 128                  # partitions
RF = 15                  # fwd aux rows: partitions 97..111
RB = 16                  # bwd aux rows: partitions 112..127
NCORES = 8
BSH = B // NCORES        # 8 examples per core
BLANK = C - 1
EPS = 1e-7               # keras backend epsilon (reference adds before log)
KAPPA = 2048.0
TSTAR = 63               # fwd covers t=0..63, bwd covers t=127..64
RENORM_F = (32, 63)   # 63: normalize U before the meet (product must not underflow)
RENORM_B = (95, 64)   # 64: normalize V in the last bwd round for the same reason
NRE = len(RENORM_F) + len(RENORM_B)
SENTINEL = 1_000_000_000
NROWS = BSH * C

# fp32 consts [128, CW] column layout
COL_IM = 0               # [0:8]     fwd init mask
COL_EM = 8               # [8:16]    bwd init (end-state indicator incl aux copies)
COL_ONE = 16             # [16:17]   fp32 ones column (final sum)
COL_BR = 17              # [17:145]  row 0 = ones row (renorm broadcast)
CW = 145
# bf16 consts [128, 2P+1]: [0:128] F, [128:256] Bw, [256] ones col (renorm sums)
CBW = 2 * P + 1

_CACHE = {}


# ---------------------------------------------------------------- host tables
def _build_core_tables(y_true, label_length):
    """Returns (offs [128,8] i32, constf [128,CW] f32, constb [128,CBW] bf16,
    overflow: bool)."""
    import ml_dtypes
    n = y_true.shape[0]
    ll = label_length.reshape(-1).astype(np.int64)
    lab = np.where(np.arange(L)[None, :] < ll[:, None], y_true.astype(np.int64), BLANK)
    ext = np.full((n, S), BLANK, dtype=np.int64)
    ext[:, 1::2] = lab

    aug = []  # (i, b, s_i): repeat at odd state s_i (skip s_i-2 -> s_i forbidden)
    for b in range(n):
        for s_i in range(3, int(min(2 * ll[b] - 1, S - 1)) + 1, 2):
            j = (s_i - 1) // 2
            if lab[b, j] == lab[b, j - 1]:
                aug.append((len(aug), b, s_i))
    overflow = len(aug) > min(RF, RB)
    aug = aug[:min(RF, RB)]

    offs = np.full((P, BSH), SENTINEL, dtype=np.int32)
    for b in range(n):
        live = int(min(2 * ll[b], S - 1))
        for s in range(live + 1):
            offs[s, b] = b * C + ext[b, s]
    for (i, b, s_i) in aug:
        offs[S + i, b] = b * C + ext[b, s_i - 2]       # fwd aux: p[s_i - 2]
        offs[S + RF + i, b] = b * C + ext[b, s_i]      # bwd aux: p[s_i]

    # forward lhsT: F[k, m] = kappa * allowed(k -> m)
    F = np.zeros((P, P), dtype=np.float64)
    for m in range(S):
        F[m, m] = 1.0
        if m >= 1:
            F[m - 1, m] = 1.0
        if m >= 2 and (m % 2 == 1):
            F[m - 2, m] = 1.0
    for (i, b, s_i) in aug:
        F[S + i, s_i] = -1.0
    for (i, b, s_i) in aug:
        F[:, S + i] = F[:, s_i - 2]

    # backward lhsT: Bw[k, m] = kappa * allowed(m -> k); G_{t-1} = Bw^T @ V_t,
    # V = G * p. Aux row i tracks V[s_i]; subtracted where the skip is forbidden.
    Bw = np.zeros((P, P), dtype=np.float64)
    for k in range(S):
        Bw[k, k] = 1.0
        if k >= 1:
            Bw[k, k - 1] = 1.0
        if k >= 2 and (k % 2 == 1):
            Bw[k, k - 2] = 1.0
    for (i, b, s_i) in aug:
        Bw[S + RF + i, s_i - 2] = -1.0
    for (i, b, s_i) in aug:
        Bw[:, S + RF + i] = Bw[:, s_i]

    constb = np.zeros((P, CBW), dtype=ml_dtypes.bfloat16)
    constb[:, 0:P] = (F * KAPPA).astype(ml_dtypes.bfloat16)
    constb[:, P:2 * P] = (Bw * KAPPA).astype(ml_dtypes.bfloat16)
    constb[:, 2 * P] = ml_dtypes.bfloat16(1.0)

    constf = np.zeros((P, CW), dtype=np.float32)
    constf[0, COL_IM:COL_IM + BSH] = 1.0
    constf[1, COL_IM:COL_IM + BSH] = 1.0
    for (i, b, s_i) in aug:
        if s_i == 3:
            constf[S + i, COL_IM + b] = 1.0
    for b in range(n):
        constf[2 * ll[b], COL_EM + b] = 1.0
        constf[2 * ll[b] - 1, COL_EM + b] = 1.0
    for (i, b, s_i) in aug:
        constf[S + RF + i, COL_EM + b] = constf[s_i, COL_EM + b]
    constf[:, COL_ONE] = 1.0
    constf[0, COL_BR:COL_BR + P] = 1.0
    return offs, constf, constb, overflow


# ---------------------------------------------------------------- host fallback
def _host_ctc(y_true_b, y_pred_b, ll_b):
    """Exact log-domain port of the reference for one example (float64)."""
    NEG = -1e30
    ll = int(ll_b)
    lab = np.where(np.arange(L) < ll, y_true_b.astype(np.int64), BLANK)
    ext = np.full((S,), BLANK, dtype=np.int64)
    ext[1::2] = lab
    lp = np.log(y_pred_b.astype(np.float64) + EPS)[:, ext]    # [T, S]
    ext_m2 = np.concatenate([[BLANK, BLANK], ext[:-2]])
    allow = (ext != BLANK) & (ext != ext_m2)
    alpha = np.where(np.arange(S) < 2, lp[0], NEG)
    for t in range(1, T):
        a0 = alpha
        a1 = np.concatenate([[NEG], alpha[:-1]])
        a2 = np.where(allow, np.concatenate([[NEG, NEG], alpha[:-2]]), NEG)
        m = np.maximum(np.maximum(a0, a1), a2)
        alpha = m + np.log(np.exp(a0 - m) + np.exp(a1 - m) + np.exp(a2 - m)) + lp[t]
    ab, al = alpha[2 * ll], alpha[2 * ll - 1]
    m = max(ab, al)
    return -(m + math.log(math.exp(ab - m) + math.exp(al - m)))


# ---------------------------------------------------------------- bass program
def _build_program():
    import concourse.bacc as bacc
    import concourse.bass as bass
    import concourse.tile as tile
    import concourse.mybir as mybir

    nc = bacc.Bacc("TRN2", target_bir_lowering=False, debug=False,
                   enable_asserts=False, num_devices=NCORES, num_swdge_queues=4)
    ypt_d = nc.dram_tensor("ypt", [NROWS, T], mybir.dt.float32, kind="ExternalInput")
    offs_d = nc.dram_tensor("offs", [P, BSH], mybir.dt.int32, kind="ExternalInput")
    cf_d = nc.dram_tensor("constf", [P, CW], mybir.dt.float32, kind="ExternalInput")
    cb_d = nc.dram_tensor("constb", [P, CBW], mybir.dt.bfloat16, kind="ExternalInput")
    loss_d = nc.dram_tensor("loss", [1, BSH], mybir.dt.float32, kind="ExternalOutput")
    warm_d = nc.dram_tensor("warm", [P, BSH], mybir.dt.float32, kind="ExternalOutput")

    fp32 = mybir.dt.float32
    bf16 = mybir.dt.bfloat16
    mult = mybir.AluOpType.mult

    with tile.TileContext(nc) as tc:
        with (
            tc.tile_pool(name="cpool", bufs=1) as cpool,
            tc.tile_pool(name="upool", bufs=2) as upool,
            tc.tile_pool(name="spool", bufs=1) as spool,
            tc.tile_pool(name="psf", bufs=2, space="PSUM") as psf,
            tc.tile_pool(name="psb", bufs=2, space="PSUM") as psb,
            tc.tile_pool(name="pss", bufs=1, space="PSUM") as pss,
        ):
            cf = cpool.tile([P, CW], fp32, tag="cf")
            nc.sync.dma_start(cf[:], cf_d[:])
            cb = cpool.tile([P, CBW], bf16, tag="cb")
            nc.sync.dma_start(cb[:], cb_d[:])
            offs = cpool.tile([P, BSH], mybir.dt.int32, tag="offs")
            nc.sync.dma_start(offs[:], offs_d[:])
            paug = cpool.tile([P, BSH, T], fp32, tag="paug")
            nc.gpsimd.memset(paug[:], 0.0)
            for b in range(BSH):
                gi = nc.gpsimd.indirect_dma_start(
                    out=paug[:, b, :],
                    out_offset=None,
                    in_=ypt_d[:],
                    in_offset=bass.IndirectOffsetOnAxis(ap=offs[:, b:b + 1], axis=0),
                    bounds_check=NROWS - 1,
                    oob_is_err=False,
                )
                if b % 4:  # spread across the 4 SWDGE queues
                    gi.ins.queue = f"qPoolDynamic{b % 4}"
            nc.vector.tensor_scalar_add(paug[:], paug[:], EPS)

            F_ap = cb[:, 0:P]
            Bw_ap = cb[:, P:2 * P]
            onesb = cb[:, 2 * P:2 * P + 1]
            norms = spool.tile([1, NRE * BSH], fp32, tag="norms")
            ri = 0

            # PE warm-up: ~20 throwaway matmuls keep the HAM activity window
            # busy during the gather phase so the DP runs at 2.4 GHz, not the
            # cold 1.2 GHz tier. Result is consumed via a dummy DRAM store so
            # nothing DCEs the chain.
            wps = psb.tile([P, BSH], fp32, tag="gp")
            for _ in range(20):
                nc.tensor.matmul(wps[:], F_ap, cb[:, 0:BSH], start=True, stop=True)
            wsb = spool.tile([P, BSH], fp32, tag="wsb")
            nc.vector.tensor_copy(wsb[:], wps[:])
            nc.sync.dma_start(warm_d[:], wsb[:])

            def renorm(Z, Zprev):
                """Divide state Z by the column sum of Zprev (the previous
                round's state, already in SBUF) - the sum matmul/recip/
                broadcast run off the serial chain; only the final multiply
                joins it. Any positive factor is exact bookkeeping: we log
                precisely the reciprocal we apply."""
                nonlocal ri
                nm = pss.tile([1, BSH], fp32, tag="sm")
                nc.tensor.matmul(nm[:], onesb, Zprev[:], start=True, stop=True)
                rrow = norms[0:1, ri * BSH:(ri + 1) * BSH]
                nc.vector.reciprocal(rrow, nm[:])
                bc = pss.tile([P, BSH], fp32, tag="bc")
                nc.tensor.matmul(bc[:], cf[0:1, COL_BR:COL_BR + P], rrow,
                                 start=True, stop=True)
                Z2 = upool.tile([P, BSH], bf16, tag="Z2")
                nc.vector.tensor_tensor(out=Z2[:], in0=Z[:], in1=bc[:], op=mult)
                ri += 1
                return Z2

            U = upool.tile([P, BSH], bf16, tag="U")
            nc.vector.tensor_tensor(
                out=U[:], in0=paug[:, :, 0], in1=cf[:, COL_IM:COL_IM + BSH], op=mult)
            gp = None  # bwd chain state (PSUM); first round uses endmask const

            Vprev = None
            for r in range(1, TSTAR + 2):
                tf_ = r           # fwd timestep this round (valid while <= TSTAR)
                tb = T - r        # bwd multiply timestep this round (127..64)
                # bwd: V = G * p[tb]; G(psum) = Bw^T V
                vin = gp[:] if gp is not None else cf[:, COL_EM:COL_EM + BSH]
                V = upool.tile([P, BSH], bf16, tag="V")
                nc.vector.tensor_tensor(out=V[:], in0=vin, in1=paug[:, :, tb], op=mult)
                if tb in RENORM_B:
                    V = renorm(V, Vprev)
                Vprev = V
                gp = psb.tile([P, BSH], fp32, tag="gp")
                nc.tensor.matmul(gp[:], Bw_ap, V[:], start=True, stop=True)
                # fwd: psum = F^T U; U = psum * p[tf]
                if tf_ <= TSTAR:
                    stp = psf.tile([P, BSH], fp32, tag="stp")
                    nc.tensor.matmul(stp[:], F_ap, U[:], start=True, stop=True)
                    Uprev = U
                    U = upool.tile([P, BSH], bf16, tag="U")
                    nc.vector.tensor_tensor(
                        out=U[:], in0=stp[:], in1=paug[:, :, tf_], op=mult)
                    if tf_ in RENORM_F:
                        U = renorm(U, Uprev)

            # meet: fin[b] = sum_s U_63[s, b] * G_63[s, b] (aux cross-terms vanish:
            # U is zero on bwd-aux rows, G zero on fwd-aux rows)
            prod = spool.tile([P, BSH], fp32, tag="prod")
            nc.vector.tensor_tensor(out=prod[:], in0=U[:], in1=gp[:], op=mult)
            fin = pss.tile([1, BSH], fp32, tag="sm")
            nc.tensor.matmul(fin[:], cf[:, COL_ONE:COL_ONE + 1], prod[:],
                             start=True, stop=True)
            lnfin = spool.tile([1, BSH], fp32, tag="lnfin")
            nc.scalar.activation(lnfin[:], fin[:], mybir.ActivationFunctionType.Ln)
            lnrec = spool.tile([1, NRE * BSH], fp32, tag="lnrec")
            nc.scalar.activation(lnrec[:], norms[:], mybir.ActivationFunctionType.Ln)
            lnrsum = spool.tile([1, BSH], fp32, tag="lnrsum")
            nc.vector.reduce_sum(
                lnrsum[:],
                lnrec[0:1, :].rearrange("p (j b) -> p b j", j=NRE),
                axis=mybir.AxisListType.X)
            total = spool.tile([1, BSH], fp32, tag="total")
            nc.vector.tensor_tensor(out=total[:], in0=lnrsum[:], in1=lnfin[:],
                                    op=mybir.AluOpType.subtract)
            loss_row = spool.tile([1, BSH], fp32, tag="loss_row")
            nc.vector.tensor_scalar_add(loss_row[:], total[:],
                                        float((T - 1) * math.log(KAPPA)))
            nc.sync.dma_start(loss_d[:], loss_row[:])

    nc.compile()
    return nc


def _get_program():
    if "nc" not in _CACHE:
        _CACHE["nc"] = _build_program()
    return _CACHE["nc"]


# ---------------------------------------------------------------- entry point
def kernel(y_true: np.ndarray, y_pred: np.ndarray, label_length: np.ndarray) -> np.ndarray:
    from concourse.bass_utils import run_bass_kernel_spmd

    y_true = np.asarray(y_true)
    y_pred = np.asarray(y_pred, dtype=np.float32)
    label_length = np.asarray(label_length)
    assert y_true.shape == (B, L) and y_pred.shape == (B, T, C), (
        f"unexpected shapes {y_true.shape} {y_pred.shape}")

    # host sharding: transpose each example's [T, C] to [C, T] so the device
    # gather reads contiguous per-class rows
    ypt = np.ascontiguousarray(y_pred.transpose(0, 2, 1))  # [B, C, T]

    in_maps = []
    fallback_cores = []
    for core in range(NCORES):
        sl = slice(core * BSH, (core + 1) * BSH)
        offs, constf, constb, overflow = _build_core_tables(y_true[sl], label_length[sl])
        if overflow:
            fallback_cores.append(core)
        in_maps.append({
            "ypt": ypt[sl].reshape(NROWS, T),
            "offs": offs,
            "constf": constf,
            "constb": constb,
        })

    nc = _get_program()
    res = run_bass_kernel_spmd(
        nc, in_maps, core_ids=list(range(NCORES)),
        trace=bool(int(os.environ.get("CTC_TRACE", "0"))),
    )
    _CACHE["last_result"] = res

    loss = np.zeros((B, 1), dtype=np.float32)
    for core in range(NCORES):
        loss[core * BSH:(core + 1) * BSH, 0] = res.results[core]["loss"][0]

    for core in fallback_cores:  # more repeats than aux rows (pathological)
        for b in range(BSH):
            g = core * BSH + b
            loss[g, 0] = _host_ctc(y_true[g], y_pred[g], label_length.reshape(-1)[g])
    return loss


# revision 11
# speedup vs baseline: 2.3468x; 1.0013x over previous
"""Trainium2 Bass kernel for CTC loss (nn_CTCLayer).

Inputs (full, unsharded):
  y_true       [64, 48]  int32  labels (blank excluded)
  y_pred       [64, 128, 4000] float32 probabilities
  label_length [64, 1]  int32
Output: loss [64, 1] float32  (= tf.keras ctc_batch_cost, input_length == T)

Strategy (pure data parallelism, 8 examples per core on 8 cores):

The CTC forward DP over S = 2L+1 = 97 extended states only ever reads
y_pred at the (<= L+1) classes present in each example's extended label
sequence, so each core gathers just those columns with one indirect DMA
per example (row-offset tables computed on the host from y_true during
sharding; y_pred is resharded host-side to [example*class, T] so class
rows are contiguous). The kernel never streams the full y_pred.

The DP runs in the scaled probability domain with states on partitions
and examples on the free axis. To halve the serial depth and keep both
the PE and DVE busy, the forward recursion (t = 0..63) and the backward
recursion (t = 127..64) run as two interleaved chains that meet at
t* = 63, where  P(l|x) = sum_s alpha_t*[s] * beta_t*[s]:

    fwd:  U_t = (F^T @ U_{t-1}) * p[:, :, t]      (matmul -> multiply)
    bwd:  V_t = G_{t} * p[:, :, t];  G_{t-1} = Bw^T @ V_t   (multiply -> matmul)

F and Bw are static per-core [128,128] bf16 matrices with entries
+-kappa (kappa = 2048 keeps products in fp32 range; exactly
representable in bf16). One sum-renormalization per chain (factors
re-applied in log space at the end) bounds the remaining drift.

Rows 97..111 (fwd) and 112..127 (bwd) are auxiliary "W" rows that
correct the forbidden skip transition s-2 -> s when ext[s] == ext[s-2]
(adjacent repeated labels): aux row i tracks the would-be-forbidden
contribution for its example only (its gathered probability row is a
copy of the relevant state's row; other examples' entries are
OOB-skipped in the gather), and the transition matrix subtracts it
where the skip is forbidden. The cancellation is bit-exact because the
aux row's matmul column is a copy of the source state's column and its
multiplier bits are identical. Pathological inputs with more repeats
than aux rows fall back to an exact host computation.

Padding states s > 2*label_length never influence the result states
(transitions are monotone in s) and their gather rows are OOB-skipped.
"""

import os
import sys
import math

import numpy as np

if "/opt/trn_rl_repo" not in sys.path:
    sys.path.insert(0, "/opt/trn_rl_repo")

# ---------------------------------------------------------------- constants
B, T, C, L = 64, 128, 4000, 48
S = 2 * L + 1            # 97 extended states
P = 128                  # partitions
RF = 15                  # fwd aux rows: partitions 97..111
RB = 16                  # bwd aux rows: partitions 112..127
NCORES = 8
BSH = B // NCORES        # 8 examples per core
BLANK = C - 1
EPS = 1e-7               # keras backend epsilon (reference adds before log)
KAPPA = 2048.0
TSTAR = 63               # fwd covers t=0..63, bwd covers t=127..64
RENORM_F = (32, 63)   # 63: normalize U before the meet (product must not underflow)
RENORM_B = (95, 64)   # 64: normalize V in the last bwd round for the same reason
NRE = len(RENORM_F) + len(RENORM_B)
SENTINEL = 1_000_000_000
NROWS = BSH * C

# fp32 consts [128, CW] column layout
COL_IM = 0               # [0:8]     fwd init mask
COL_EM = 8               # [8:16]    bwd init (end-state indicator incl aux copies)
COL_ONE = 16             # [16:17]   fp32 ones column (final sum)
COL_BR = 17              # [17:145]  row 0 = ones row (renorm broadcast)
CW = 145
# bf16 consts [128, 2P+1]: [0:128] F, [128:256] Bw, [256] ones col (renorm sums)
CBW = 2 * P + 1

_CACHE = {}


# ---------------------------------------------------------------- host tables
def _build_core_tables(y_true, label_length):
    """Returns (offs [128,8] i32, constf [128,CW] f32, constb [128,CBW] bf16,
    overflow: bool)."""
    import ml_dtypes
    n = y_true.shape[0]
    ll = label_length.reshape(-1).astype(np.int64)
    lab = np.where(np.arange(L)[None, :] < ll[:, None], y_true.astype(np.int64), BLANK)
    ext = np.full((n, S), BLANK, dtype=np.int64)
    ext[:, 1::2] = lab

    aug = []  # (i, b, s_i): repeat at odd state s_i (skip s_i-2 -> s_i forbidden)
    for b in range(n):
        for s_i in range(3, int(min(2 * ll[b] - 1, S - 1)) + 1, 2):
            j = (s_i - 1) // 2
            if lab[b, j] == lab[b, j - 1]:
                aug.append((len(aug), b, s_i))
    overflow = len(aug) > min(RF, RB)
    aug = aug[:min(RF, RB)]

    offs = np.full((P, BSH), SENTINEL, dtype=np.int32)
    for b in range(n):
        live = int(min(2 * ll[b], S - 1))
        for s in range(live + 1):
            offs[s, b] = b * C + ext[b, s]
    for (i, b, s_i) in aug:
        offs[S + i, b] = b * C + ext[b, s_i - 2]       # fwd aux: p[s_i - 2]
        offs[S + RF + i, b] = b * C + ext[b, s_i]      # bwd aux: p[s_i]

    # forward lhsT: F[k, m] = kappa * allowed(k -> m)
    F = np.zeros((P, P), dtype=np.float64)
    for m in range(S):
        F[m, m] = 1.0
        if m >= 1:
            F[m - 1, m] = 1.0
        if m >= 2 and (m % 2 == 1):
            F[m - 2, m] = 1.0
    for (i, b, s_i) in aug:
        F[S + i, s_i] = -1.0
    for (i, b, s_i) in aug:
        F[:, S + i] = F[:, s_i - 2]

    # backward lhsT: Bw[k, m] = kappa * allowed(m -> k); G_{t-1} = Bw^T @ V_t,
    # V = G * p. Aux row i tracks V[s_i]; subtracted where the skip is forbidden.
    Bw = np.zeros((P, P), dtype=np.float64)
    for k in range(S):
        Bw[k, k] = 1.0
        if k >= 1:
            Bw[k, k - 1] = 1.0
        if k >= 2 and (k % 2 == 1):
            Bw[k, k - 2] = 1.0
    for (i, b, s_i) in aug:
        Bw[S + RF + i, s_i - 2] = -1.0
    for (i, b, s_i) in aug:
        Bw[:, S + RF + i] = Bw[:, s_i]

    constb = np.zeros((P, CBW), dtype=ml_dtypes.bfloat16)
    constb[:, 0:P] = (F * KAPPA).astype(ml_dtypes.bfloat16)
    constb[:, P:2 * P] = (Bw * KAPPA).astype(ml_dtypes.bfloat16)
    constb[:, 2 * P] = ml_dtypes.bfloat16(1.0)

    constf = np.zeros((P, CW), dtype=np.float32)
    constf[0, COL_IM:COL_IM + BSH] = 1.0
    constf[1, COL_IM:COL_IM + BSH] = 1.0
    for (i, b, s_i) in aug:
        if s_i == 3:
            constf[S + i, COL_IM + b] = 1.0
    for b in range(n):
        constf[2 * ll[b], COL_EM + b] = 1.0
        constf[2 * ll[b] - 1, COL_EM + b] = 1.0
    for (i, b, s_i) in aug:
        constf[S + RF + i, COL_EM + b] = constf[s_i, COL_EM + b]
    constf[:, COL_ONE] = 1.0
    constf[0, COL_BR:COL_BR + P] = 1.0
    return offs, constf, constb, overflow


# ---------------------------------------------------------------- host fallback
def _host_ctc(y_true_b, y_pred_b, ll_b):
    """Exact log-domain port of the reference for one example (float64)."""
    NEG = -1e30
    ll = int(ll_b)
    lab = np.where(np.arange(L) < ll, y_true_b.astype(np.int64), BLANK)
    ext = np.full((S,), BLANK, dtype=np.int64)
    ext[1::2] = lab
    lp = np.log(y_pred_b.astype(np.float64) + EPS)[:, ext]    # [T, S]
    ext_m2 = np.concatenate([[BLANK, BLANK], ext[:-2]])
    allow = (ext != BLANK) & (ext != ext_m2)
    alpha = np.where(np.arange(S) < 2, lp[0], NEG)
    for t in range(1, T):
        a0 = alpha
        a1 = np.concatenate([[NEG], alpha[:-1]])
        a2 = np.where(allow, np.concatenate([[NEG, NEG], alpha[:-2]]), NEG)
        m = np.maximum(np.maximum(a0, a1), a2)
        alpha = m + np.log(np.exp(a0 - m) + np.exp(a1 - m) + np.exp(a2 - m)) + lp[t]
    ab, al = alpha[2 * ll], alpha[2 * ll - 1]
    m = max(ab, al)
    return -(m + math.log(math.exp(ab - m) + math.exp(al - m)))


# ---------------------------------------------------------------- bass program
def _build_program():
    import concourse.bacc as bacc
    import concourse.bass as bass
    import concourse.tile as tile
    import concourse.mybir as mybir

    nc = bacc.Bacc("TRN2", target_bir_lowering=False, debug=False,
                   enable_asserts=False, num_devices=NCORES, num_swdge_queues=4)
    ypt_d = nc.dram_tensor("ypt", [NROWS, T], mybir.dt.float32, kind="ExternalInput")
    offs_d = nc.dram_tensor("offs", [P, BSH], mybir.dt.int32, kind="ExternalInput")
    cf_d = nc.dram_tensor("constf", [P, CW], mybir.dt.float32, kind="ExternalInput")
    cb_d = nc.dram_tensor("constb", [P, CBW], mybir.dt.bfloat16, kind="ExternalInput")
    loss_d = nc.dram_tensor("loss", [1, BSH], mybir.dt.float32, kind="ExternalOutput")
    warm_d = nc.dram_tensor("warm", [P, BSH], mybir.dt.float32, kind="ExternalOutput")

    fp32 = mybir.dt.float32
    bf16 = mybir.dt.bfloat16
    mult = mybir.AluOpType.mult

    with tile.TileContext(nc) as tc:
        with (
            tc.tile_pool(name="cpool", bufs=1) as cpool,
            tc.tile_pool(name="upool", bufs=2) as upool,
            tc.tile_pool(name="spool", bufs=1) as spool,
            tc.tile_pool(name="psf", bufs=2, space="PSUM") as psf,
            tc.tile_pool(name="psb", bufs=2, space="PSUM") as psb,
            tc.tile_pool(name="pss", bufs=1, space="PSUM") as pss,
        ):
            cf = cpool.tile([P, CW], fp32, tag="cf")
            nc.sync.dma_start(cf[:], cf_d[:])
            cb = cpool.tile([P, CBW], bf16, tag="cb")
            nc.sync.dma_start(cb[:], cb_d[:])
            offs = cpool.tile([P, BSH], mybir.dt.int32, tag="offs")
            nc.sync.dma_start(offs[:], offs_d[:])
            paug = cpool.tile([P, BSH, T], fp32, tag="paug")
            nc.gpsimd.memset(paug[:], 0.0)
            for b in range(BSH):
                gi = nc.gpsimd.indirect_dma_start(
                    out=paug[:, b, :],
                    out_offset=None,
                    in_=ypt_d[:],
                    in_offset=bass.IndirectOffsetOnAxis(ap=offs[:, b:b + 1], axis=0),
                    bounds_check=NROWS - 1,
                    oob_is_err=False,
                )
                if b % 4:  # spread across the 4 SWDGE queues
                    gi.ins.queue = f"qPoolDynamic{b % 4}"
            nc.vector.tensor_scalar_add(paug[:], paug[:], EPS)

            F_ap = cb[:, 0:P]
            Bw_ap = cb[:, P:2 * P]
            onesb = cb[:, 2 * P:2 * P + 1]
            norms = spool.tile([1, NRE * BSH], fp32, tag="norms")
            ri = 0

            # PE warm-up: throwaway matmuls keep the HAM activity window
            # busy (~3.5us of sustained PE work) during the gather phase so
            # the DP runs at 2.4 GHz, not the cold 1.2 GHz tier. Result is
            # consumed via a dummy DRAM store so nothing DCEs the chain.
            wps = psb.tile([P, BSH], fp32, tag="gp")
            for _ in range(130):
                nc.tensor.matmul(wps[:], F_ap, cb[:, 0:BSH], start=True, stop=True)
            wsb = spool.tile([P, BSH], fp32, tag="wsb")
            nc.vector.tensor_copy(wsb[:], wps[:])
            nc.sync.dma_start(warm_d[:], wsb[:])

            def renorm(Z, Zprev):
                """Divide state Z by the column sum of Zprev (the previous
                round's state, already in SBUF) - the sum matmul/recip/
                broadcast run off the serial chain; only the final multiply
                joins it. Any positive factor is exact bookkeeping: we log
                precisely the reciprocal we apply."""
                nonlocal ri
                nm = pss.tile([1, BSH], fp32, tag="sm")
                nc.tensor.matmul(nm[:], onesb, Zprev[:], start=True, stop=True)
                rrow = norms[0:1, ri * BSH:(ri + 1) * BSH]
                nc.vector.reciprocal(rrow, nm[:])
                bc = pss.tile([P, BSH], fp32, tag="bc")
                nc.tensor.matmul(bc[:], cf[0:1, COL_BR:COL_BR + P], rrow,
                                 start=True, stop=True)
                Z2 = upool.tile([P, BSH], bf16, tag="Z2")
                nc.vector.tensor_tensor(out=Z2[:], in0=Z[:], in1=bc[:], op=mult)
                ri += 1
                return Z2

            U = upool.tile([P, BSH], bf16, tag="U")
            nc.vector.tensor_tensor(
                out=U[:], in0=paug[:, :, 0], in1=cf[:, COL_IM:COL_IM + BSH], op=mult)
            gp = None  # bwd chain state (PSUM); first round uses endmask const

            Vprev = None
            for r in range(1, TSTAR + 2):
                tf_ = r           # fwd timestep this round (valid while <= TSTAR)
                tb = T - r        # bwd multiply timestep this round (127..64)
                # bwd: V = G * p[tb]; G(psum) = Bw^T V
                vin = gp[:] if gp is not None else cf[:, COL_EM:COL_EM + BSH]
                V = upool.tile([P, BSH], bf16, tag="V")
                nc.vector.tensor_tensor(out=V[:], in0=vin, in1=paug[:, :, tb], op=mult)
                if tb in RENORM_B:
                    V = renorm(V, Vprev)
                Vprev = V
                gp = psb.tile([P, BSH], fp32, tag="gp")
                nc.tensor.matmul(gp[:], Bw_ap, V[:], start=True, stop=True)
                # fwd: psum = F^T U; U = psum * p[tf]
                if tf_ <= TSTAR:
                    stp = psf.tile([P, BSH], fp32, tag="stp")
                    nc.tensor.matmul(stp[:], F_ap, U[:], start=True, stop=True)
                    Uprev = U
                    U = upool.tile([P, BSH], bf16, tag="U")
                    nc.vector.tensor_tensor(
                        out=U[:], in0=stp[:], in1=paug[:, :, tf_], op=mult)
                    if tf_ in RENORM_F:
                        U = renorm(U, Uprev)

            # meet: fin[b] = sum_s U_63[s, b] * G_63[s, b] (aux cross-terms vanish:
            # U is zero on bwd-aux rows, G zero on fwd-aux rows)
            prod = spool.tile([P, BSH], fp32, tag="prod")
            nc.vector.tensor_tensor(out=prod[:], in0=U[:], in1=gp[:], op=mult)
            fin = pss.tile([1, BSH], fp32, tag="sm")
            nc.tensor.matmul(fin[:], cf[:, COL_ONE:COL_ONE + 1], prod[:],
                             start=True, stop=True)
            lnfin = spool.tile([1, BSH], fp32, tag="lnfin")
            nc.scalar.activation(lnfin[:], fin[:], mybir.ActivationFunctionType.Ln)
            lnrec = spool.tile([1, NRE * BSH], fp32, tag="lnrec")
            nc.scalar.activation(lnrec[:], norms[:], mybir.ActivationFunctionType.Ln)
            lnrsum = spool.tile([1, BSH], fp32, tag="lnrsum")
            nc.vector.reduce_sum(
                lnrsum[:],
                lnrec[0:1, :].rearrange("p (j b) -> p b j", j=NRE),
                axis=mybir.AxisListType.X)
            total = spool.tile([1, BSH], fp32, tag="total")
            nc.vector.tensor_tensor(out=total[:], in0=lnrsum[:], in1=lnfin[:],
                                    op=mybir.AluOpType.subtract)
            loss_row = spool.tile([1, BSH], fp32, tag="loss_row")
            nc.vector.tensor_scalar_add(loss_row[:], total[:],
                                        float((T - 1) * math.log(KAPPA)))
            nc.sync.dma_start(loss_d[:], loss_row[:])

    nc.compile()
    return nc


def _get_program():
    if "nc" not in _CACHE:
        _CACHE["nc"] = _build_program()
    return _CACHE["nc"]


# ---------------------------------------------------------------- entry point
def kernel(y_true: np.ndarray, y_pred: np.ndarray, label_length: np.ndarray) -> np.ndarray:
    from concourse.bass_utils import run_bass_kernel_spmd

    y_true = np.asarray(y_true)
    y_pred = np.asarray(y_pred, dtype=np.float32)
    label_length = np.asarray(label_length)
    assert y_true.shape == (B, L) and y_pred.shape == (B, T, C), (
        f"unexpected shapes {y_true.shape} {y_pred.shape}")

    # host sharding: transpose each example's [T, C] to [C, T] so the device
    # gather reads contiguous per-class rows
    ypt = np.ascontiguousarray(y_pred.transpose(0, 2, 1))  # [B, C, T]

    in_maps = []
    fallback_cores = []
    for core in range(NCORES):
        sl = slice(core * BSH, (core + 1) * BSH)
        offs, constf, constb, overflow = _build_core_tables(y_true[sl], label_length[sl])
        if overflow:
            fallback_cores.append(core)
        in_maps.append({
            "ypt": ypt[sl].reshape(NROWS, T),
            "offs": offs,
            "constf": constf,
            "constb": constb,
        })

    nc = _get_program()
    res = run_bass_kernel_spmd(
        nc, in_maps, core_ids=list(range(NCORES)),
        trace=bool(int(os.environ.get("CTC_TRACE", "0"))),
    )
    _CACHE["last_result"] = res

    loss = np.zeros((B, 1), dtype=np.float32)
    for core in range(NCORES):
        loss[core * BSH:(core + 1) * BSH, 0] = res.results[core]["loss"][0]

    for core in fallback_cores:  # more repeats than aux rows (pathological)
        for b in range(BSH):
            g = core * BSH + b
            loss[g, 0] = _host_ctc(y_true[g], y_pred[g], label_length.reshape(-1)[g])
    return loss
